# revision 15
# baseline (speedup 1.0000x reference)
"""Trainium2 Bass kernel: 3x3 'same' cross-correlation on a [1,1,8192,8192] fp32 image.

Strategy (8 NeuronCores, row-sharded, memory-bound target):
  - Host: pad image rows/cols by 1, cast to bf16 (tolerance is 2e-2; bf16
    round-off contributes ~4e-3 worst-case), shard into 8 overlapping
    [1026, 8194] row-shards (1 halo row each side). Kernel values arrive at
    trace time, so the Bass program is specialized to the nonzero taps of
    the 3x3 kernel.
  - Device (per core): for each tile of R output rows, load a single
    [R+nb, Wp] bf16 image tile A (nb = kernel row-span - 1). One banded
    matmul per nonzero kernel *column* (the band shifts across partitions
    for the row taps; the kernel-column offset is a free-dim shift on the
    rhs AP). All taps accumulate in PSUM; drains PSUM->SBUF(bf16) alternate
    between DVE (tensor_copy) and ACT (activation Copy) so neither engine
    binds. Output DMAs straight from SBUF as bf16.
  - HBM traffic is ~2B/px each way (half of fp32): ~34 MB/core, the
    roofline for this memory-bound problem at ~358 GB/s per core.
"""

import numpy as np
import ml_dtypes

import concourse.bass as bass
import concourse.mybir as mybir
from concourse import bacc
from concourse import bass_utils
from concourse import tile

H = 8192
W = 8192
N_CORES = 8
RPC = H // N_CORES  # rows per core

F32 = mybir.dt.float32
BF16 = mybir.dt.bfloat16


def _nonzero_taps(kern3: np.ndarray):
    """[(j, i, w)] for nonzero entries of the 3x3 kernel."""
    return [
        (j, i, float(kern3[j, i]))
        for j in range(kern3.shape[0])
        for i in range(kern3.shape[1])
        if kern3[j, i] != 0.0
    ]


def _band_matrix(col_taps, jmin, k_rows, out_rows):
    """lhsT [k_rows, out_rows] with B[k, p] = w for each (j, w) in col_taps
    where k = p + (j - jmin). matmul computes psum[p,:] = sum_k B[k,p]*A[k,:]."""
    B = np.zeros((k_rows, out_rows), dtype=np.float32)
    for j, w in col_taps:
        d = j - jmin
        for p in range(out_rows):
            k = p + d
            if 0 <= k < k_rows:
                B[k, p] = w
    return B


def build_program(kern3: np.ndarray, *, width=W, rpc=RPC,
                  mm_cols=512, a_bufs=5, out_bufs=3, psum_bufs=8):
    """Build the per-core Bass program. Shard layout: S[r] =
    padded_image[core_row0 + r], r in [0, rpc+2); out rows r in [0, rpc).

    The image tile a carries one zero column of padding on each side, so
    every tap's rhs slice [c0+i, c0+i+mm_cols) is in range and every matmul
    is full-width."""
    taps = _nonzero_taps(kern3)
    assert taps, "all-zero kernel handled host-side"

    jmin = min(j for j, _, _ in taps)
    jmax = max(j for j, _, _ in taps)
    nb = jmax - jmin  # extra rows of A needed beyond R
    R = 128 - nb  # output rows per tile

    # group taps by kernel column
    cols = {}
    for j, i, w in taps:
        cols.setdefault(i, []).append((j, w))
    col_ids = sorted(cols.keys())

    nc = bacc.Bacc("TRN2", target_bir_lowering=False, debug=False,
                   num_devices=N_CORES)
    s_in = nc.dram_tensor("shard", [rpc + 2, width + 2], BF16, kind="ExternalInput").ap()
    out_d = nc.dram_tensor("out", [rpc, width], BF16, kind="ExternalOutput").ap()
    bands_in = nc.dram_tensor(
        "bands", [len(col_ids), 128, 128], BF16, kind="ExternalInput"
    ).ap()

    # tiles of output rows
    tiles = []
    t = 0
    while t < rpc:
        r = min(R, rpc - t)
        tiles.append((t, r))
        t += r

    # A trailing thin tile (krows <= 32) is packed 4x across PE row-groups
    # via tile_position: 4 concurrent matmuls on disjoint 32-row strips.
    pack_last = len(tiles) > 1 and (tiles[-1][1] + nb) <= 32
    if pack_last:
        thin_bands_in = nc.dram_tensor(
            "thin_bands", [len(col_ids), 128, 128], BF16, kind="ExternalInput"
        ).ap()

    wp = width + 2  # padded tile width
    n_q = width // mm_cols
    assert width % mm_cols == 0

    with tile.TileContext(nc) as tc:
        with (
            tc.tile_pool(name="bandp", bufs=1) as bandp,
            tc.tile_pool(name="ap", bufs=a_bufs) as apool,
            tc.tile_pool(name="op", bufs=out_bufs) as opool,
            tc.tile_pool(name="pp", bufs=psum_bufs, space="PSUM") as ppool,
        ):
            # band loads go on the ACT ring so the first image load can
            # issue immediately on the Sync ring
            band_tiles = {}
            for ci, i in enumerate(col_ids):
                bt = bandp.tile([128, 128], BF16, tag=f"band{ci}")
                nc.scalar.dma_start(out=bt, in_=bands_in[ci])
                band_tiles[i] = bt
            thin_band_tiles = {}
            if pack_last:
                for ci, i in enumerate(col_ids):
                    tbt = bandp.tile([128, 128], BF16, tag=f"tband{ci}")
                    nc.scalar.dma_start(out=tbt, in_=thin_bands_in[ci])
                    thin_band_tiles[i] = tbt

            # superchunks: one band's weights are loaded once and reused
            # across `sc` psum chunks before switching bands.
            sc = min(psum_bufs, n_q)
            assert n_q % sc == 0
            half = (n_q // 2) * mm_cols  # column split point for DMA halves

            eng_i = 0
            for ti, (t0, rt) in enumerate(tiles):
                krows = rt + nb  # contraction rows for this tile
                packed = pack_last and ti == len(tiles) - 1
                a = apool.tile([128, wp], BF16, tag="a")
                # The first tile's load gates the whole pipeline: split it
                # into slices so the matmuls start as soon as the first
                # slice lands. Later tiles load in one DMA (prefetched).
                if ti == 0:
                    splits = [0, 1026, 2050, half + 2, wp]
                    for s0, s1 in zip(splits, splits[1:]):
                        nc.sync.dma_start(
                            out=a[0:krows, s0:s1],
                            in_=s_in[t0 + jmin: t0 + jmin + krows, s0:s1],
                        )
                elif packed:
                    # replicate the thin tile's rows into all 4 row-groups
                    for g4 in range(4):
                        nc.sync.dma_start(
                            out=a[32 * g4: 32 * g4 + krows, :],
                            in_=s_in[t0 + jmin: t0 + jmin + krows, :],
                        )
                else:
                    nc.sync.dma_start(
                        out=a[0:krows, :],
                        in_=s_in[t0 + jmin: t0 + jmin + krows, :],
                    )
                o = opool.tile([128, width], BF16, tag="o")

                for g in range(n_q // sc):
                    ps_tiles = [
                        ppool.tile([128, mm_cols], F32, tag="ps", name=f"ps{ci}")
                        for ci in range(sc)
                    ]
                    chunk_order = (0, 2, 4, 6, 1, 3, 5, 7) if packed else range(sc)
                    for ii, i in enumerate(col_ids):
                        for ci in chunk_order:
                            q0 = (g * sc + ci) * mm_cols
                            # rhs cols [q0+i, q0+i+mm_cols) in padded coords
                            if packed:
                                g4 = ci // 2
                                nc.tensor.matmul(
                                    out=ps_tiles[ci][0:128, :],
                                    lhsT=thin_band_tiles[i][
                                        32 * g4: 32 * g4 + krows, 0:128
                                    ],
                                    rhs=a[32 * g4: 32 * g4 + krows,
                                          q0 + i:q0 + i + mm_cols],
                                    start=(ii == 0),
                                    stop=(ii == len(col_ids) - 1),
                                    tile_position=(32 * g4, 0),
                                )
                            else:
                                nc.tensor.matmul(
                                    out=ps_tiles[ci][0:128, :],
                                    lhsT=band_tiles[i][0:krows, 0:128],
                                    rhs=a[0:krows, q0 + i:q0 + i + mm_cols],
                                    start=(ii == 0),
                                    stop=(ii == len(col_ids) - 1),
                                )
                    # drain psum -> out sbuf (bf16), alternating engines
                    for ci in range(sc):
                        q0 = (g * sc + ci) * mm_cols
                        if eng_i % 2 == 0:
                            nc.vector.tensor_copy(
                                o[0:rt, q0:q0 + mm_cols], ps_tiles[ci][0:rt, :]
                            )
                        else:
                            nc.scalar.copy(
                                o[0:rt, q0:q0 + mm_cols], ps_tiles[ci][0:rt, :]
                            )
                        eng_i += 1
                    # store this group's columns as soon as they are drained.
                    # Issued from the ACT ring (nc.scalar) so a store waiting
                    # on drains can't head-of-line-block the next tile's load
                    # on the Sync ring.
                    g0 = g * sc * mm_cols
                    g1 = (g + 1) * sc * mm_cols
                    nc.scalar.dma_start(
                        out=out_d[t0: t0 + rt, g0:g1], in_=o[0:rt, g0:g1]
                    )

    nc.compile()

    meta = {
        "bands": np.stack([
            _band_matrix(cols[i], jmin, 128, 128) for i in col_ids
        ]).astype(ml_dtypes.bfloat16),
    }
    if pack_last:
        kr = tiles[-1][1] + nb
        tb = []
        for i in col_ids:
            B = _band_matrix(cols[i], jmin, kr, 128)  # [kr, 128]
            full = np.zeros((128, 128), dtype=np.float32)
            for g4 in range(4):
                full[32 * g4: 32 * g4 + kr, :] = B
            tb.append(full)
        meta["thin_bands"] = np.stack(tb).astype(ml_dtypes.bfloat16)
    return nc, meta


def kernel(image: np.ndarray, kernel: np.ndarray) -> np.ndarray:
    image = np.asarray(image)
    kernel = np.asarray(kernel, dtype=np.float32)
    img = np.ascontiguousarray(image.reshape(H, W).astype(np.float32))

    if not np.any(kernel):
        return np.zeros_like(image, dtype=np.float32).reshape(image.shape)

    nc, meta = build_program(kernel)

    padded = np.pad(img, ((1, 1), (1, 1))).astype(ml_dtypes.bfloat16)
    in_maps = []
    for c in range(N_CORES):
        m = {
            "shard": np.ascontiguousarray(padded[c * RPC: c * RPC + RPC + 2]),
            "bands": meta["bands"],
        }
        if "thin_bands" in meta:
            m["thin_bands"] = meta["thin_bands"]
        in_maps.append(m)

    res = bass_utils.run_bass_kernel_spmd(nc, in_maps, core_ids=list(range(N_CORES)))
    out = np.concatenate(
        [np.asarray(r["out"]).astype(np.float32) for r in res.results], axis=0
    )
    return out.reshape(image.shape)


# revision 19
# speedup vs baseline: 1.1292x; 1.1292x over previous
"""Trainium2 Bass kernel: 3x3 'same' cross-correlation on a [1,1,8192,8192] fp32 image.

Strategy (8 NeuronCores, row-sharded, memory-bound target):
  - Host: pad image rows/cols by 1, cast to bf16 (tolerance is 2e-2; bf16
    round-off contributes ~4e-3 worst-case), shard into 8 overlapping
    [1026, 8194] row-shards (1 halo row each side). Kernel values arrive at
    trace time, so the Bass program is specialized to the nonzero taps of
    the 3x3 kernel.
  - Device (per core): for each tile of R output rows, load a single
    [R+nb, Wp] bf16 image tile A (nb = kernel row-span - 1). One banded
    matmul per nonzero kernel *column* (the band shifts across partitions
    for the row taps; the kernel-column offset is a free-dim shift on the
    rhs AP). All taps accumulate in PSUM; drains PSUM->SBUF(bf16) alternate
    between DVE (tensor_copy) and ACT (activation Copy) so neither engine
    binds. Output DMAs straight from SBUF as bf16.
  - HBM traffic is ~2B/px each way (half of fp32): ~34 MB/core, the
    roofline for this memory-bound problem at ~358 GB/s per core.
"""

import numpy as np
import ml_dtypes

import concourse.bass as bass
import concourse.mybir as mybir
from concourse import bacc
from concourse import bass_utils
from concourse import tile

H = 8192
W = 8192
N_CORES = 8
RPC = H // N_CORES  # rows per core

F32 = mybir.dt.float32
BF16 = mybir.dt.bfloat16


def _nonzero_taps(kern3: np.ndarray):
    """[(j, i, w)] for nonzero entries of the 3x3 kernel."""
    return [
        (j, i, float(kern3[j, i]))
        for j in range(kern3.shape[0])
        for i in range(kern3.shape[1])
        if kern3[j, i] != 0.0
    ]


def _band_matrix(col_taps, jmin, k_rows, out_rows):
    """lhsT [k_rows, out_rows] with B[k, p] = w for each (j, w) in col_taps
    where k = p + (j - jmin). matmul computes psum[p,:] = sum_k B[k,p]*A[k,:]."""
    B = np.zeros((k_rows, out_rows), dtype=np.float32)
    for j, w in col_taps:
        d = j - jmin
        for p in range(out_rows):
            k = p + d
            if 0 <= k < k_rows:
                B[k, p] = w
    return B


def build_program(kern3: np.ndarray, *, width=W, rpc=RPC,
                  mm_cols=512, a_bufs=5, out_bufs=4, psum_bufs=8):
    """Build the per-core Bass program. Shard layout: S[r] =
    padded_image[core_row0 + r], r in [0, rpc+2); out rows r in [0, rpc).

    The image tile a carries one zero column of padding on each side, so
    every tap's rhs slice [c0+i, c0+i+mm_cols) is in range and every matmul
    is full-width."""
    taps = _nonzero_taps(kern3)
    assert taps, "all-zero kernel handled host-side"

    jmin = min(j for j, _, _ in taps)
    jmax = max(j for j, _, _ in taps)
    nb = jmax - jmin  # extra rows of A needed beyond R
    R = 128 - nb  # output rows per tile

    # group taps by kernel column
    cols = {}
    for j, i, w in taps:
        cols.setdefault(i, []).append((j, w))
    col_ids = sorted(cols.keys())

    nc = bacc.Bacc("TRN2", target_bir_lowering=False, debug=False,
                   num_devices=N_CORES)
    s_in = nc.dram_tensor("shard", [rpc + 2, width + 2], BF16, kind="ExternalInput").ap()
    out_d = nc.dram_tensor("out", [rpc, width], BF16, kind="ExternalOutput").ap()
    bands_in = nc.dram_tensor(
        "bands", [len(col_ids), 128, 128], BF16, kind="ExternalInput"
    ).ap()

    # tiles of output rows
    tiles = []
    t = 0
    while t < rpc:
        r = min(R, rpc - t)
        tiles.append((t, r))
        t += r

    # A trailing thin tile (krows <= 32) is packed 4x across PE row-groups
    # via tile_position: 4 concurrent matmuls on disjoint 32-row strips.
    pack_last = len(tiles) > 1 and (tiles[-1][1] + nb) <= 32
    if pack_last:
        thin_bands_in = nc.dram_tensor(
            "thin_bands", [len(col_ids), 128, 128], BF16, kind="ExternalInput"
        ).ap()

    wp = width + 2  # padded tile width
    n_q = width // mm_cols
    assert width % mm_cols == 0

    with tile.TileContext(nc) as tc:
        with (
            tc.tile_pool(name="bandp", bufs=1) as bandp,
            tc.tile_pool(name="ap", bufs=a_bufs) as apool,
            tc.tile_pool(name="op", bufs=out_bufs) as opool,
            tc.tile_pool(name="pp", bufs=psum_bufs, space="PSUM") as ppool,
        ):
            # band loads go on the ACT ring so the first image load can
            # issue immediately on the Sync ring
            band_tiles = {}
            for ci, i in enumerate(col_ids):
                bt = bandp.tile([128, 128], BF16, tag=f"band{ci}")
                nc.scalar.dma_start(out=bt, in_=bands_in[ci])
                band_tiles[i] = bt
            thin_band_tiles = {}
            if pack_last:
                for ci, i in enumerate(col_ids):
                    tbt = bandp.tile([128, 128], BF16, tag=f"tband{ci}")
                    nc.scalar.dma_start(out=tbt, in_=thin_bands_in[ci])
                    thin_band_tiles[i] = tbt

            # superchunks: one band's weights are loaded once and reused
            # across `sc` psum chunks before switching bands.
            sc = min(psum_bufs, n_q)
            assert n_q % sc == 0
            half = (n_q // 2) * mm_cols  # column split point for DMA halves

            eng_i = 0
            for ti, (t0, rt) in enumerate(tiles):
                krows = rt + nb  # contraction rows for this tile
                packed = pack_last and ti == len(tiles) - 1
                a = apool.tile([128, wp], BF16, tag="a")
                # The first tile's load gates the whole pipeline: split it
                # into slices so the matmuls start as soon as the first
                # slice lands. Later tiles load in one DMA (prefetched).
                if ti == 0:
                    splits = [0, 516, 2050, half + 2, wp]
                    for s0, s1 in zip(splits, splits[1:]):
                        nc.sync.dma_start(
                            out=a[0:krows, s0:s1],
                            in_=s_in[t0 + jmin: t0 + jmin + krows, s0:s1],
                        )
                elif packed:
                    # replicate the thin tile's rows into all 4 row-groups
                    for g4 in range(4):
                        nc.sync.dma_start(
                            out=a[32 * g4: 32 * g4 + krows, :],
                            in_=s_in[t0 + jmin: t0 + jmin + krows, :],
                        )
                else:
                    nc.sync.dma_start(
                        out=a[0:krows, :],
                        in_=s_in[t0 + jmin: t0 + jmin + krows, :],
                    )
                o = opool.tile([128, width], BF16, tag="o")

                for g in range(n_q // sc):
                    ps_tiles = [
                        ppool.tile([128, mm_cols], F32, tag="ps", name=f"ps{ci}")
                        for ci in range(sc)
                    ]
                    chunk_order = (0, 2, 4, 6, 1, 3, 5, 7) if packed else range(sc)
                    for ii, i in enumerate(col_ids):
                        for ci in chunk_order:
                            q0 = (g * sc + ci) * mm_cols
                            # rhs cols [q0+i, q0+i+mm_cols) in padded coords
                            if packed:
                                g4 = ci // 2
                                nc.tensor.matmul(
                                    out=ps_tiles[ci][0:128, :],
                                    lhsT=thin_band_tiles[i][
                                        32 * g4: 32 * g4 + krows, 0:128
                                    ],
                                    rhs=a[32 * g4: 32 * g4 + krows,
                                          q0 + i:q0 + i + mm_cols],
                                    start=(ii == 0),
                                    stop=(ii == len(col_ids) - 1),
                                    tile_position=(32 * g4, 0),
                                )
                            else:
                                nc.tensor.matmul(
                                    out=ps_tiles[ci][0:128, :],
                                    lhsT=band_tiles[i][0:krows, 0:128],
                                    rhs=a[0:krows, q0 + i:q0 + i + mm_cols],
                                    start=(ii == 0),
                                    stop=(ii == len(col_ids) - 1),
                                )
                    # drain psum -> out sbuf (bf16), alternating engines
                    for ci in range(sc):
                        q0 = (g * sc + ci) * mm_cols
                        if eng_i % 2 == 0:
                            nc.vector.tensor_copy(
                                o[0:rt, q0:q0 + mm_cols], ps_tiles[ci][0:rt, :]
                            )
                        else:
                            nc.scalar.copy(
                                o[0:rt, q0:q0 + mm_cols], ps_tiles[ci][0:rt, :]
                            )
                        eng_i += 1
                    # store this group's columns as soon as they are drained.
                    # Issued from the ACT ring (nc.scalar) so a store waiting
                    # on drains can't head-of-line-block the next tile's load
                    # on the Sync ring. For the last two tiles there are no
                    # loads left to block, so alternate rings to pipeline the
                    # ~2.5us completion receipts at the kernel tail.
                    g0 = g * sc * mm_cols
                    g1 = (g + 1) * sc * mm_cols
                    if ti == len(tiles) - 1:
                        eng = nc.sync if (g % 2 == 0) else nc.scalar
                    else:
                        eng = nc.scalar
                    eng.dma_start(
                        out=out_d[t0: t0 + rt, g0:g1], in_=o[0:rt, g0:g1]
                    )

    nc.compile()

    meta = {
        "bands": np.stack([
            _band_matrix(cols[i], jmin, 128, 128) for i in col_ids
        ]).astype(ml_dtypes.bfloat16),
    }
    if pack_last:
        kr = tiles[-1][1] + nb
        tb = []
        for i in col_ids:
            B = _band_matrix(cols[i], jmin, kr, 128)  # [kr, 128]
            full = np.zeros((128, 128), dtype=np.float32)
            for g4 in range(4):
                full[32 * g4: 32 * g4 + kr, :] = B
            tb.append(full)
        meta["thin_bands"] = np.stack(tb).astype(ml_dtypes.bfloat16)
    return nc, meta


def kernel(image: np.ndarray, kernel: np.ndarray) -> np.ndarray:
    image = np.asarray(image)
    kernel = np.asarray(kernel, dtype=np.float32)
    img = np.ascontiguousarray(image.reshape(H, W).astype(np.float32))

    if not np.any(kernel):
        return np.zeros_like(image, dtype=np.float32).reshape(image.shape)

    nc, meta = build_program(kernel)

    padded = np.pad(img, ((1, 1), (1, 1))).astype(ml_dtypes.bfloat16)
    in_maps = []
    for c in range(N_CORES):
        m = {
            "shard": np.ascontiguousarray(padded[c * RPC: c * RPC + RPC + 2]),
            "bands": meta["bands"],
        }
        if "thin_bands" in meta:
            m["thin_bands"] = meta["thin_bands"]
        in_maps.append(m)

    res = bass_utils.run_bass_kernel_spmd(nc, in_maps, core_ids=list(range(N_CORES)))
    out = np.concatenate(
        [np.asarray(r["out"]).astype(np.float32) for r in res.results], axis=0
    )
    return out.reshape(image.shape)
